# revision 10
# baseline (speedup 1.0000x reference)
"""CrossAttentionHead kernel for 8x TRN2 NeuronCores (Bass/Tile).

Reference computation (all fp32):
    Q = q @ Wq.T + bq          # [S, DQ]      S=4096, DQ=1024
    K = k @ Wk.T + bk          # [S, DK]      DK=4096
    V = v @ Wv.T + bv          # [S, DK]
    num  = Q.T @ K             # [DQ, DK]
    attn = softmax(num / sqrt(DK), axis=-1)
    out  = attn @ V            # [DQ, DK]   (contraction uses S == DK)

Algebraic restructure (the key optimization): K and V are never
materialized.  With M = Q.T @ k  ([DQ, DIN]):
    num  = M @ Wk.T + (Q.T @ 1) x bk          (bias is rank-1)
    out  = (attn @ v) @ Wv.T + bv             (attn rows sum to 1)
This cuts the FLOPs 4x (146 -> 36.5 GFLOP total) and shards cleanly by
ROWS of the output: core i owns DQ rows [i*128, (i+1)*128).  Per core:
    Qi   = q @ Wq_i.T + bq_i                  # [S, 128]
    MT   = k.T-chunks @ Qi                    # M_i.T  [DIN, 128]
    numT = Wk-chunks @ MT + bk x r            # [DK, 128], r = colsum(Qi)
    ET   = exp(numT / 64)                     # softmax numerator (no max
                                              # subtraction: |num/64| < ~3)
    s    = ET.T @ 1                           # [128, 1] denominators
    GT   = v.T-chunks @ ET                    # (E @ v).T  [DIN, 128]
    out  = (GT.T @ Wv.T) * (1/s)              # [128, DK]
Every matmul lands in its natural [K-on-partitions] layout -- zero
transposes.  No cross-core communication; the host concatenates the 8
row shards and adds bv.

Precision: HBM loads are float8_e3m4 (q, k, v, Wk.T, Wv.T; weights
pre-scaled x64 so entries ~U(-1,1) stay in e3m4's normal range),
intermediates bf16, PSUM/bias fp32.  Numpy-sim max rel err ~1.3e-2
(gate 2e-2).  e3m4 halves DMA bytes (21 MB/core) which would otherwise
be the bottleneck (360 GB/s/core aggregate in the cost model).
"""

import threading

import numpy as np
import ml_dtypes

S = 4096
DIN = 1024
DQ = 1024
DK = 4096
NCORES = 8
P = 128
QSH = DQ // NCORES          # 128: per-core shard of DQ rows
ST = S // P                 # 32 s-tiles
CT = DIN // P               # 8 contraction chunks over DIN
KT = DK // P                # 32 k-tiles
GRP = 4                     # DMA grouping: 4 tiles per transfer
WS = 64.0                   # weight prescale (e3m4 range/subnormal fix)
SCALE = 1.0 / (64.0 * WS * WS)   # exp arg = numT_scaled * SCALE

DEBUG = False   # adds intermediate-dump outputs to the module

_lock = threading.Lock()
_cache = {}


def _build_module():
    import concourse.bacc as bacc
    import concourse.mybir as mybir
    import concourse.tile as tile

    bf16 = mybir.dt.bfloat16
    f32 = mybir.dt.float32
    e3 = mybir.dt.float8e3
    Exp = mybir.ActivationFunctionType.Exp
    Copy = mybir.ActivationFunctionType.Copy

    nc = bacc.Bacc(
        "TRN2", target_bir_lowering=False, debug=False, num_devices=NCORES
    )

    # [g, din-part, st-in-g, ct, s]: lhsT tiles for P1
    qT = nc.dram_tensor("qT", [ST // GRP, P, GRP, CT, P], e3,
                        kind="ExternalInput").ap()
    # [g, s-part, st-in-g, din]: lhsT tiles for P2 (k natural row-major)
    kT = nc.dram_tensor("kT", [ST // GRP, P, GRP, DIN], e3,
                        kind="ExternalInput").ap()
    # [g, k-part, kt-in-g, din]: lhsT tiles for P5 (v natural row-major)
    vT = nc.dram_tensor("vT", [KT // GRP, P, GRP, DIN], e3,
                        kind="ExternalInput").ap()
    # [g, din-part, kt-in-g, ct, k]: lhsT tiles for P3
    wkT = nc.dram_tensor("wkT", [KT // GRP, P, GRP, CT, P], e3,
                         kind="ExternalInput").ap()
    # [b, din-part, ct, 512]: moving tiles for P6
    wvT = nc.dram_tensor("wvT", [CT, P, CT, 512], e3,
                         kind="ExternalInput").ap()
    # [din-part, ct, q]: moving tile for P1 (per-core shard, x WS)
    wqT = nc.dram_tensor("wqT", [P, CT, P], bf16, kind="ExternalInput").ap()
    bq = nc.dram_tensor("bq", [GRP * P], bf16, kind="ExternalInput").ap()  # x WS, tiled 4x
    bk = nc.dram_tensor("bk", [DK], bf16, kind="ExternalInput").ap()    # x WS
    ident = nc.dram_tensor("ident", [P, P], bf16, kind="ExternalInput").ap()
    p_out = nc.dram_tensor("p_out", [P, CT, 512], bf16,
                           kind="ExternalOutput").ap()
    if DEBUG:
        qi_dbg = nc.dram_tensor("qi_dbg", [P, ST * P], bf16,
                                kind="ExternalOutput").ap()
        mt_dbg = nc.dram_tensor("mt_dbg", [P, DIN], bf16,
                                kind="ExternalOutput").ap()
        r_dbg = nc.dram_tensor("r_dbg", [1, P], bf16,
                               kind="ExternalOutput").ap()
        et_dbg = nc.dram_tensor("et_dbg", [P, KT * P], bf16,
                                kind="ExternalOutput").ap()
        gt_dbg = nc.dram_tensor("gt_dbg", [P, DIN], bf16,
                                kind="ExternalOutput").ap()
        rec_dbg = nc.dram_tensor("rec_dbg", [P, 1], f32,
                                 kind="ExternalOutput").ap()

    def ts(i, sz):
        return slice(i * sz, (i + 1) * sz)

    W4 = GRP * P  # 512: one PSUM bank of fp32

    with tile.TileContext(nc) as tc:
        with tc.tile_pool(name="persist", bufs=1) as persist:
            WqT_sb = persist.tile([P, CT, P], bf16)
            Bq = persist.tile([P, W4], bf16)          # bq_i tiled 4x
            bk_sb = persist.tile([32, DK], bf16)   # row 0 = bk*WS, rows 1-31 zero
                                                   # (K=1 matmuls round tile_size
                                                   # up to 32 partitions on HW)
            ones1 = persist.tile([P, 1], bf16)
            onesS = persist.tile([P, 1], bf16)
            Qi_sb = persist.tile([P, ST * P], bf16)   # 8 KB/part
            MT_sb = persist.tile([P, DIN], bf16)
            ET_sb = persist.tile([P, KT * P], bf16)   # 8 KB/part
            GT_sb = persist.tile([P, DIN], bf16)
            r_sb = persist.tile([32, P], bf16)     # row 0 = r, rows 1-31 zero
            zpad = persist.tile([32, 512], bf16)   # zeros: opens shared PSUM banks
            id_sb = persist.tile([P, P], bf16)     # identity: r column->row
            rc_sb = persist.tile([P, 1], bf16)
            rec_sb = persist.tile([P, 1], f32)
            nc.vector.memset(ones1[:], 1.0)
            nc.vector.memset(onesS[:], WS)
            nc.vector.memset(bk_sb[:], 0.0)
            nc.vector.memset(r_sb[:], 0.0)
            nc.vector.memset(zpad[:], 0.0)

            # All load DMAs go on the sync (SP) queue in need-order; the
            # pool buffer rotation (bufs=3) head-of-line-blocks the queue
            # so later tensors cannot steal DMA bandwidth early.
            nc.sync.dma_start(WqT_sb[:, 0:2], wqT[:, 0:2])
            nc.sync.dma_start(WqT_sb[:, 2:CT], wqT[:, 2:CT])

            qpool = tc.alloc_tile_pool(name="qp", bufs=3)
            kpool = tc.alloc_tile_pool(name="kp", bufs=3)
            qips = tc.alloc_tile_pool(name="qips", bufs=2, space="PSUM")
            mtps = tc.alloc_tile_pool(name="mtps", bufs=1, space="PSUM")
            mt_ps = [mtps.tile([P, W4], f32, name=f"mt{h}") for h in range(2)]
            r_ps = mtps.tile([1, P], f32, name="rps")
            rc_ps = mtps.tile([P, 1], f32, name="rcps")

            # ---- phase A: Qi projection (P1) + MT = k.T @ Qi (P2) ----
            # PSUM accumulators are packed 4-per-bank; evictions and the
            # bias add run on whole [P, 512] banks.  A start=True matmul
            # marks its entire 2KB zero region (= the bank) pending-zero,
            # so packed banks are opened ONCE by a zeroing matmul and all
            # real accumulation runs start=False with a single final stop.
            # P1/P2 are software-pipelined one group apart so P2(g) never
            # waits on the DVE eviction of Qi(g).
            for h in range(2):
                nc.tensor.matmul(mt_ps[h][:], zpad[:, 0:P], zpad[:],
                                 start=True, stop=False)

            def do_p1(g, qt):
                qp = qips.tile([P, W4], f32, tag="qip", name=f"qp{g}")
                for j in range(GRP):
                    for ct in range(CT):
                        nc.tensor.matmul(
                            qp[:, ts(j, P)], qt[:, j, ct, :], WqT_sb[:, ct, :],
                            start=(ct == 0), stop=(ct == CT - 1),
                        )
                nc.vector.tensor_add(Qi_sb[:, ts(g, W4)], qp[:], Bq[:])

            def do_p2(g, kt):
                for j in range(GRP):
                    st = g * GRP + j
                    for ct in range(CT):
                        nc.tensor.matmul(
                            mt_ps[ct // GRP][:, ts(ct % GRP, P)],
                            kt[:, j, ts(ct, P)], Qi_sb[:, ts(st, P)],
                            start=False,
                            stop=(st == ST - 1 and ct % GRP == GRP - 1),
                        )

            kts = {}
            for g in range(ST // GRP):
                qt = qpool.tile([P, GRP, CT, P], e3, tag="qt", name=f"qt{g}")
                if g == 0:
                    # split the first load so P1 starts ~3us earlier
                    nc.sync.dma_start(qt[:, 0:2], qT[0][:, 0:2])
                    nc.sync.dma_start(Bq[:], bq.unsqueeze(0).to_broadcast((P, W4)))
                    nc.sync.dma_start(qt[:, 2:4], qT[0][:, 2:4])
                else:
                    nc.sync.dma_start(qt[:], qT[g])
                kt = kpool.tile([P, GRP, DIN], e3, tag="kt", name=f"kt{g}")
                nc.sync.dma_start(kt[:], kT[g])
                if g == 0:
                    nc.sync.dma_start(bk_sb[0:1, :], bk.unsqueeze(0))
                    nc.sync.dma_start(id_sb[:], ident)
                kts[g] = kt
                do_p1(g, qt)
                if g >= 1:
                    do_p2(g - 1, kts[g - 1])
            do_p2(ST // GRP - 1, kts[ST // GRP - 1])
            # r = colsum(Qi), accumulated as a column (N=1 matmuls are ~free)
            # then flipped to a row with one identity matmul.
            for st in range(ST):
                nc.tensor.matmul(
                    rc_ps[:], Qi_sb[:, ts(st, P)], ones1[:],
                    start=(st == 0), stop=(st == ST - 1),
                )
            for h in range(2):
                nc.vector.tensor_copy(MT_sb[:, ts(h, W4)], mt_ps[h][:])
            nc.vector.tensor_copy(rc_sb[:], rc_ps[:])
            nc.tensor.matmul(r_ps[:], rc_sb[:], id_sb[:], start=True, stop=True)
            nc.vector.tensor_copy(r_sb[0:1, :], r_ps[:])
            mtps.release()
            qips.release()
            kpool.release()
            qpool.release()

            # ---- phase B: numT (P3) -> exp -> GT (P5) + s, per k-tile ----
            # Pipelined one group apart so GT(g) never waits on the exp of
            # its own group.  Within a numT region the rank-1 bias (bk x r)
            # runs LAST so r's eviction stays off the critical path.
            wkpool = tc.alloc_tile_pool(name="wkp", bufs=3)
            vpool = tc.alloc_tile_pool(name="vp", bufs=3)
            ntps = tc.alloc_tile_pool(name="ntps", bufs=3, space="PSUM")
            gtps = tc.alloc_tile_pool(name="gtps", bufs=1, space="PSUM")
            gt_ps = [gtps.tile([P, W4], f32, name=f"gt{h}") for h in range(2)]
            s_ps = gtps.tile([P, 1], f32, name="sps")
            for h in range(2):
                nc.tensor.matmul(gt_ps[h][:], zpad[:, 0:P], zpad[:],
                                 start=True, stop=False)

            def do_nt(g, wk):
                ntb = ntps.tile([P, W4], f32, tag="nt", name=f"nt{g}")
                for j in range(GRP):
                    ktile = g * GRP + j
                    for ct in range(CT):
                        nc.tensor.matmul(
                            ntb[:, ts(j, P)], wk[:, j, ct, :],
                            MT_sb[:, ts(ct, P)],
                            start=(ct == 0), stop=False,
                        )
                    nc.tensor.matmul(
                        ntb[:, ts(j, P)], bk_sb[0:32, ts(ktile, P)],
                        r_sb[0:32, :], start=False, stop=True,
                    )
                nc.scalar.activation(
                    ET_sb[:, ts(g, W4)], ntb[:], Exp, scale=SCALE,
                )

            def do_gt(g, vt):
                for j in range(GRP):
                    ktile = g * GRP + j
                    for ct in range(CT):
                        nc.tensor.matmul(
                            gt_ps[ct // GRP][:, ts(ct % GRP, P)],
                            vt[:, j, ts(ct, P)], ET_sb[:, ts(ktile, P)],
                            start=False,
                            stop=(ktile == KT - 1 and ct % GRP == GRP - 1),
                        )
                    nc.tensor.matmul(
                        s_ps[:], ET_sb[:, ts(ktile, P)], onesS[:],
                        start=(ktile == 0), stop=(ktile == KT - 1),
                    )

            vts = {}
            for g in range(KT // GRP):
                wk = wkpool.tile([P, GRP, CT, P], e3, tag="wk", name=f"wk{g}")
                nc.sync.dma_start(wk[:], wkT[g])
                vt = vpool.tile([P, GRP, DIN], e3, tag="vt", name=f"vt{g}")
                nc.sync.dma_start(vt[:], vT[g])
                vts[g] = vt
                do_nt(g, wk)
                if g >= 1:
                    do_gt(g - 1, vts[g - 1])
            # final group bank-major: bank 0 finishes (and evicts on DVE)
            # while bank 1's matmuls still run; bank 1 evicts on ACT.
            gl = KT // GRP - 1
            vt = vts[gl]
            for h in range(2):
                for j in range(GRP):
                    ktile = gl * GRP + j
                    for ct in range(4 * h, 4 * h + 4):
                        nc.tensor.matmul(
                            gt_ps[h][:, ts(ct % GRP, P)],
                            vt[:, j, ts(ct, P)], ET_sb[:, ts(ktile, P)],
                            start=False,
                            stop=(j == GRP - 1 and ct % GRP == GRP - 1),
                        )
                if h == 0:
                    nc.vector.tensor_copy(GT_sb[:, ts(0, W4)], gt_ps[0][:])
            for j in range(GRP):
                ktile = gl * GRP + j
                nc.tensor.matmul(
                    s_ps[:], ET_sb[:, ts(ktile, P)], onesS[:],
                    start=(ktile == 0), stop=(ktile == KT - 1),
                )
            nc.scalar.activation(GT_sb[:, ts(1, W4)], gt_ps[1][:], Copy)
            nc.vector.reciprocal(rec_sb[:], s_ps[:])
            gtps.release()
            ntps.release()
            vpool.release()
            wkpool.release()

            if DEBUG:
                nc.scalar.dma_start(qi_dbg, Qi_sb[:])
                nc.scalar.dma_start(mt_dbg, MT_sb[:])
                nc.scalar.dma_start(r_dbg, r_sb[0:1, :])
                nc.scalar.dma_start(et_dbg, ET_sb[:])
                nc.scalar.dma_start(gt_dbg, GT_sb[:])
                nc.scalar.dma_start(rec_dbg, rec_sb[:])

            # ---- phase C: out = (GT.T @ WvT) * (1/s) ----
            with tc.tile_pool(name="wvp", bufs=3) as wvpool, \
                 tc.tile_pool(name="ops", bufs=4, space="PSUM") as ops, \
                 tc.tile_pool(name="otp", bufs=4) as otp:
                for b in range(CT):
                    wv = wvpool.tile([P, CT, 512], e3, tag="wv")
                    nc.sync.dma_start(wv[:], wvT[b])
                    op = ops.tile([P, 512], f32, tag="op")
                    ot = otp.tile([P, 512], bf16, tag="ot")
                    if b == CT - 1:
                        # halve the final bank end-to-end so the last
                        # ACT+DMA chain starts ~0.9us earlier
                        for hh in range(2):
                            for ct in range(CT):
                                nc.tensor.matmul(
                                    op[:, ts(hh, 256)], GT_sb[:, ts(ct, P)],
                                    wv[:, ct, ts(hh, 256)],
                                    start=(ct == 0), stop=(ct == CT - 1),
                                )
                            nc.scalar.activation(
                                ot[:, ts(hh, 256)], op[:, ts(hh, 256)],
                                Copy, scale=rec_sb[:])
                            nc.scalar.dma_start(
                                p_out[:, b, ts(hh, 256)], ot[:, ts(hh, 256)])
                    else:
                        for ct in range(CT):
                            nc.tensor.matmul(
                                op[:], GT_sb[:, ts(ct, P)], wv[:, ct, :],
                                start=(ct == 0), stop=(ct == CT - 1),
                            )
                        nc.scalar.activation(ot[:], op[:], Copy, scale=rec_sb[:])
                        nc.scalar.dma_start(p_out[:, b, :], ot[:])

    nc.compile()
    return nc


def _e3(a):
    return np.ascontiguousarray(a.astype(ml_dtypes.float8_e3m4))


def _bf16(a):
    return np.ascontiguousarray(a.astype(ml_dtypes.bfloat16))


def make_in_maps(q, k, v, Wq, bq, Wk, bk, Wv, bv):
    """Host-side shard + layout prep. Returns per-core input dicts."""
    f32 = np.float32
    q, k, v = (np.asarray(a, f32) for a in (q, k, v))
    Wq, bq, Wk, bk, Wv = (np.asarray(a, f32) for a in (Wq, bq, Wk, bk, Wv))

    # qT: q.T [din, s] -> [g, din-part, st-in-g, ct, s-in-tile]
    qt = q.T.reshape(CT, P, ST, P).transpose(2, 1, 0, 3)      # [st, dp, ct, s]
    qT = _e3(qt.reshape(ST // GRP, GRP, P, CT, P).transpose(0, 2, 1, 3, 4))
    # kT / vT: row-major [g, s-part, st-in-g, din]
    kT = _e3(k.reshape(ST // GRP, GRP, P, DIN).transpose(0, 2, 1, 3))
    vT = _e3(v.reshape(KT // GRP, GRP, P, DIN).transpose(0, 2, 1, 3))
    # wkT: Wk.T*WS [din, k] -> [g, din-part, kt-in-g, ct, k-in-tile]
    wkt = (Wk.T * WS).reshape(CT, P, KT, P).transpose(2, 1, 0, 3)
    wkT = _e3(wkt.reshape(KT // GRP, GRP, P, CT, P).transpose(0, 2, 1, 3, 4))
    # wvT: Wv.T*WS [din, dk] -> [b, din-part, ct, 512]
    wvT = _e3((Wv.T * WS).reshape(CT, P, CT, 512).transpose(2, 1, 0, 3))
    bkr = _bf16(bk * WS)
    idm = _bf16(np.eye(P, dtype=f32))

    in_maps = []
    for i in range(NCORES):
        sl = slice(i * QSH, (i + 1) * QSH)
        wq_i = _bf16((Wq[sl].T * WS).reshape(CT, P, QSH).transpose(1, 0, 2))
        bq_i = _bf16(np.tile(bq[sl] * WS, GRP))
        in_maps.append({
            "qT": qT, "kT": kT, "vT": vT, "wkT": wkT, "wvT": wvT,
            "wqT": wq_i, "bq": bq_i, "bk": bkr, "ident": idm,
        })
    return in_maps


def combine(results, bv):
    """Host-side unshard: concatenate row shards, add bv."""
    out = np.concatenate(
        [r["p_out"].reshape(QSH, DK).astype(np.float32) for r in results],
        axis=0,
    )
    return out + np.asarray(bv, np.float32)[None, :]


def get_nc():
    with _lock:
        if "nc" not in _cache:
            _cache["nc"] = _build_module()
        return _cache["nc"]


def _run_spmd(in_maps):
    """Execute on the 8 NeuronCores.

    Under axon this mirrors bass_utils.run_bass_kernel_spmd's redirect
    (bass2jax.run_bass_via_pjrt) with two wall-clock fixes: the jitted
    executable is cached across calls, and core-replicated inputs use a
    replicated sharding instead of an 8x host-side concat.
    """
    from concourse._compat import axon_active
    from concourse import bass_utils

    nc = get_nc()
    if not axon_active():
        res = bass_utils.run_bass_kernel_spmd(nc, in_maps, list(range(NCORES)))
        return res.results
    r = _get_axon_runner(nc)
    return r.unpack(r.fn(*r.pack(in_maps)))


_SHARED = ("qT", "kT", "vT", "wkT", "wvT", "bk", "ident")  # identical on every core


class _AxonRunner:
    def __init__(self, nc, donate):
        import jax
        import numpy as _np
        from jax.sharding import Mesh, PartitionSpec, NamedSharding
        from jax.experimental.shard_map import shard_map
        import concourse.mybir as mybir
        from concourse import bass2jax

        bass2jax.install_neuronx_cc_hook()
        pname = nc.partition_id_tensor.name if nc.partition_id_tensor else None

        self.in_names, self.out_names, out_avals, self.zero_outs = [], [], [], []
        for alloc in nc.m.functions[0].allocations:
            if not isinstance(alloc, mybir.MemoryLocationSet):
                continue
            name = alloc.memorylocations[0].name
            if alloc.kind == "ExternalInput":
                if name != pname:
                    self.in_names.append(name)
            elif alloc.kind == "ExternalOutput":
                shape = tuple(alloc.tensor_shape)
                dtype = mybir.dt.np(alloc.dtype)
                self.out_names.append(name)
                out_avals.append(jax.core.ShapedArray(shape, dtype))
                self.zero_outs.append(_np.zeros(shape, dtype))
        self.out_avals = out_avals
        n_params = len(self.in_names)
        n_outs = len(out_avals)
        all_in_names = list(self.in_names) + list(self.out_names)
        if pname is not None:
            all_in_names.append(pname)

        def _body(*args):
            operands = list(args)
            if pname is not None:
                operands.append(bass2jax.partition_id_tensor())
            outs = bass2jax._bass_exec_p.bind(
                *operands,
                out_avals=tuple(out_avals),
                in_names=tuple(all_in_names),
                out_names=tuple(self.out_names),
                lowering_input_output_aliases=(),
                sim_require_finite=True,
                sim_require_nnan=True,
                nc=nc,
            )
            return tuple(outs)

        devices = jax.devices()[:NCORES]
        self.mesh = Mesh(_np.asarray(devices), ("core",))
        rep, sh = PartitionSpec(), PartitionSpec("core")
        self.in_specs = tuple(
            rep if n in _SHARED else sh for n in self.in_names
        ) + (sh,) * n_outs
        out_specs = (sh,) * n_outs
        donate_argnums = (
            tuple(range(n_params, n_params + n_outs)) if donate else ()
        )
        self.fn = jax.jit(
            shard_map(_body, mesh=self.mesh, in_specs=self.in_specs,
                      out_specs=out_specs, check_rep=False),
            donate_argnums=donate_argnums, keep_unused=True,
        )
        self._jax = jax
        self._NamedSharding = NamedSharding

    def pack(self, in_maps):
        import numpy as _np
        args = []
        for name in self.in_names:
            if name in _SHARED:
                args.append(_np.asarray(in_maps[0][name]))
            else:
                args.append(
                    _np.concatenate(
                        [_np.asarray(m[name]) for m in in_maps], axis=0)
                )
        for z in self.zero_outs:
            args.append(_np.zeros((NCORES * z.shape[0], *z.shape[1:]), z.dtype))
        return args

    def to_device(self, args):
        """Pre-place packed args with their shardings (for timing loops)."""
        return [
            self._jax.device_put(
                a, self._NamedSharding(self.mesh, spec))
            for a, spec in zip(args, self.in_specs)
        ]

    def unpack(self, out_arrs):
        import numpy as _np
        return [
            {
                name: _np.asarray(out_arrs[i]).reshape(
                    NCORES, *self.out_avals[i].shape)[c]
                for i, name in enumerate(self.out_names)
            }
            for c in range(NCORES)
        ]


def _get_axon_runner(nc, donate=False):
    """Cached executable; donate=False keeps output operands reusable
    across calls (legal here: the kernel writes every output element,
    so nothing reads the pre-zeroed buffers)."""
    key = ("runner", donate)
    with _lock:
        if key in _cache:
            return _cache[key]
    runner = _AxonRunner(nc, donate)
    with _lock:
        _cache[key] = runner
    return runner


def kernel(q, k, v, Wq, bq, Wk, bk, Wv, bv):
    q, k, v, Wq, bq, Wk, bk, Wv, bv = (
        np.asarray(a) for a in (q, k, v, Wq, bq, Wk, bk, Wv, bv))
    in_maps = make_in_maps(q, k, v, Wq, bq, Wk, bk, Wv, bv)
    results = _run_spmd(in_maps)
    return combine(results, np.asarray(bv))


# revision 11
# speedup vs baseline: 1.0010x; 1.0010x over previous
"""CrossAttentionHead kernel for 8x TRN2 NeuronCores (Bass/Tile).

Reference computation (all fp32):
    Q = q @ Wq.T + bq          # [S, DQ]      S=4096, DQ=1024
    K = k @ Wk.T + bk          # [S, DK]      DK=4096
    V = v @ Wv.T + bv          # [S, DK]
    num  = Q.T @ K             # [DQ, DK]
    attn = softmax(num / sqrt(DK), axis=-1)
    out  = attn @ V            # [DQ, DK]   (contraction uses S == DK)

Algebraic restructure (the key optimization): K and V are never
materialized.  With M = Q.T @ k  ([DQ, DIN]):
    num  = M @ Wk.T + (Q.T @ 1) x bk          (bias is rank-1)
    out  = (attn @ v) @ Wv.T + bv             (attn rows sum to 1)
This cuts the FLOPs 4x (146 -> 36.5 GFLOP total) and shards cleanly by
ROWS of the output: core i owns DQ rows [i*128, (i+1)*128).  Per core:
    Qi   = q @ Wq_i.T + bq_i                  # [S, 128]
    MT   = k.T-chunks @ Qi                    # M_i.T  [DIN, 128]
    numT = Wk-chunks @ MT + bk x r            # [DK, 128], r = colsum(Qi)
    ET   = exp(numT / 64)                     # softmax numerator (no max
                                              # subtraction: |num/64| < ~3)
    s    = ET.T @ 1                           # [128, 1] denominators
    GT   = v.T-chunks @ ET                    # (E @ v).T  [DIN, 128]
    out  = (GT.T @ Wv.T) * (1/s)              # [128, DK]
Every matmul lands in its natural [K-on-partitions] layout -- zero
transposes.  No cross-core communication; the host concatenates the 8
row shards and adds bv.

Precision: HBM loads are float8_e3m4 (q, k, v, Wk.T, Wv.T; weights
pre-scaled x64 so entries ~U(-1,1) stay in e3m4's normal range),
intermediates bf16, PSUM/bias fp32.  Numpy-sim max rel err ~1.3e-2
(gate 2e-2).  e3m4 halves DMA bytes (21 MB/core) which would otherwise
be the bottleneck (360 GB/s/core aggregate in the cost model).
"""

import threading

import numpy as np
import ml_dtypes

S = 4096
DIN = 1024
DQ = 1024
DK = 4096
NCORES = 8
P = 128
QSH = DQ // NCORES          # 128: per-core shard of DQ rows
ST = S // P                 # 32 s-tiles
CT = DIN // P               # 8 contraction chunks over DIN
KT = DK // P                # 32 k-tiles
GRP = 4                     # DMA grouping: 4 tiles per transfer
WS = 64.0                   # weight prescale (e3m4 range/subnormal fix)
SCALE = 1.0 / (64.0 * WS * WS)   # exp arg = numT_scaled * SCALE

DEBUG = False   # adds intermediate-dump outputs to the module

_lock = threading.Lock()
_cache = {}


def _build_module():
    import concourse.bacc as bacc
    import concourse.mybir as mybir
    import concourse.tile as tile

    bf16 = mybir.dt.bfloat16
    f32 = mybir.dt.float32
    e3 = mybir.dt.float8e3
    Exp = mybir.ActivationFunctionType.Exp
    Copy = mybir.ActivationFunctionType.Copy

    nc = bacc.Bacc(
        "TRN2", target_bir_lowering=False, debug=False, num_devices=NCORES
    )

    # [g, din-part, st-in-g, ct, s]: lhsT tiles for P1
    qT = nc.dram_tensor("qT", [ST // GRP, P, GRP, CT, P], e3,
                        kind="ExternalInput").ap()
    # [g, s-part, st-in-g, din]: lhsT tiles for P2 (k natural row-major)
    kT = nc.dram_tensor("kT", [ST // GRP, P, GRP, DIN], e3,
                        kind="ExternalInput").ap()
    # [g, k-part, kt-in-g, din]: lhsT tiles for P5 (v natural row-major)
    vT = nc.dram_tensor("vT", [KT // GRP, P, GRP, DIN], e3,
                        kind="ExternalInput").ap()
    # [g, din-part, kt-in-g, ct, k]: lhsT tiles for P3
    wkT = nc.dram_tensor("wkT", [KT // GRP, P, GRP, CT, P], e3,
                         kind="ExternalInput").ap()
    # [b, din-part, ct, 512]: moving tiles for P6
    wvT = nc.dram_tensor("wvT", [CT, P, CT, 512], e3,
                         kind="ExternalInput").ap()
    # [din-part, ct, q]: moving tile for P1 (per-core shard, x WS)
    wqT = nc.dram_tensor("wqT", [P, CT, P], bf16, kind="ExternalInput").ap()
    bq = nc.dram_tensor("bq", [GRP * P], bf16, kind="ExternalInput").ap()  # x WS, tiled 4x
    bk = nc.dram_tensor("bk", [DK], bf16, kind="ExternalInput").ap()    # x WS
    p_out = nc.dram_tensor("p_out", [P, CT, 512], bf16,
                           kind="ExternalOutput").ap()
    if DEBUG:
        qi_dbg = nc.dram_tensor("qi_dbg", [P, ST * P], bf16,
                                kind="ExternalOutput").ap()
        mt_dbg = nc.dram_tensor("mt_dbg", [P, DIN], bf16,
                                kind="ExternalOutput").ap()
        r_dbg = nc.dram_tensor("r_dbg", [1, P], bf16,
                               kind="ExternalOutput").ap()
        et_dbg = nc.dram_tensor("et_dbg", [P, KT * P], bf16,
                                kind="ExternalOutput").ap()
        gt_dbg = nc.dram_tensor("gt_dbg", [P, DIN], bf16,
                                kind="ExternalOutput").ap()
        rec_dbg = nc.dram_tensor("rec_dbg", [P, 1], f32,
                                 kind="ExternalOutput").ap()

    def ts(i, sz):
        return slice(i * sz, (i + 1) * sz)

    W4 = GRP * P  # 512: one PSUM bank of fp32

    with tile.TileContext(nc) as tc:
        with tc.tile_pool(name="persist", bufs=1) as persist:
            WqT_sb = persist.tile([P, CT, P], bf16)
            Bq = persist.tile([P, W4], bf16)          # bq_i tiled 4x
            bk_sb = persist.tile([32, DK], bf16)   # row 0 = bk*WS, rows 1-31 zero
                                                   # (K=1 matmuls round tile_size
                                                   # up to 32 partitions on HW)
            ones1 = persist.tile([P, 1], bf16)
            onesS = persist.tile([P, 1], bf16)
            Qi_sb = persist.tile([P, ST * P], bf16)   # 8 KB/part
            MT_sb = [persist.tile([P, W4], bf16, name=f"MT{h}")
                     for h in range(2)]
            ET_sb = persist.tile([P, KT * P], bf16)   # 8 KB/part
            GT_sb = [persist.tile([P, W4], bf16, name=f"GT{h}")
                     for h in range(2)]
            r_sb = persist.tile([32, P], bf16)     # row 0 = r, rows 1-31 zero
            zpad = persist.tile([32, 512], bf16)   # zeros: opens shared PSUM banks
            rec_sb = persist.tile([P, 1], f32)
            nc.vector.memset(ones1[:], 1.0)
            nc.vector.memset(onesS[:], WS)
            nc.vector.memset(bk_sb[:], 0.0)
            nc.vector.memset(r_sb[:], 0.0)
            nc.vector.memset(zpad[:], 0.0)

            # All load DMAs go on the sync (SP) queue in need-order; the
            # pool buffer rotation (bufs=3) head-of-line-blocks the queue
            # so later tensors cannot steal DMA bandwidth early.
            nc.sync.dma_start(WqT_sb[:, 0:2], wqT[:, 0:2])
            nc.sync.dma_start(WqT_sb[:, 2:CT], wqT[:, 2:CT])

            qpool = tc.alloc_tile_pool(name="qp", bufs=3)
            kpool = tc.alloc_tile_pool(name="kp", bufs=3)
            qips = tc.alloc_tile_pool(name="qips", bufs=2, space="PSUM")
            mtps = tc.alloc_tile_pool(name="mtps", bufs=1, space="PSUM")
            mt_ps = [mtps.tile([P, W4], f32, name=f"mt{h}") for h in range(2)]
            r_ps = mtps.tile([1, P], f32, name="rps")

            # ---- phase A: Qi projection (P1) + MT = k.T @ Qi (P2) ----
            # PSUM accumulators are packed 4-per-bank; evictions and the
            # bias add run on whole [P, 512] banks.  A start=True matmul
            # marks its entire 2KB zero region (= the bank) pending-zero,
            # so packed banks are opened ONCE by a zeroing matmul and all
            # real accumulation runs start=False with a single final stop.
            # P1/P2 are software-pipelined one group apart so P2(g) never
            # waits on the DVE eviction of Qi(g).
            for h in range(2):
                nc.tensor.matmul(mt_ps[h][:], zpad[:, 0:P], zpad[:],
                                 start=True, stop=False)

            def do_p1(g, qt):
                qp = qips.tile([P, W4], f32, tag="qip", name=f"qp{g}")
                for j in range(GRP):
                    for ct in range(CT):
                        nc.tensor.matmul(
                            qp[:, ts(j, P)], qt[:, j, ct, :], WqT_sb[:, ct, :],
                            start=(ct == 0), stop=(ct == CT - 1),
                        )
                nc.vector.tensor_add(Qi_sb[:, ts(g, W4)], qp[:], Bq[:])

            def do_p2(g, kt):
                for j in range(GRP):
                    st = g * GRP + j
                    for ct in range(CT):
                        nc.tensor.matmul(
                            mt_ps[ct // GRP][:, ts(ct % GRP, P)],
                            kt[:, j, ts(ct, P)], Qi_sb[:, ts(st, P)],
                            start=False,
                            stop=(st == ST - 1 and ct % GRP == GRP - 1),
                        )

            kts = {}
            for g in range(ST // GRP):
                qt = qpool.tile([P, GRP, CT, P], e3, tag="qt", name=f"qt{g}")
                if g == 0:
                    # split the first load so P1 starts ~3us earlier
                    nc.sync.dma_start(qt[:, 0:2], qT[0][:, 0:2])
                    nc.sync.dma_start(Bq[:], bq.unsqueeze(0).to_broadcast((P, W4)))
                    nc.sync.dma_start(qt[:, 2:4], qT[0][:, 2:4])
                else:
                    nc.sync.dma_start(qt[:], qT[g])
                kt = kpool.tile([P, GRP, DIN], e3, tag="kt", name=f"kt{g}")
                nc.sync.dma_start(kt[:], kT[g])
                if g == 0:
                    nc.sync.dma_start(bk_sb[0:1, :], bk.unsqueeze(0))
                kts[g] = kt
                do_p1(g, qt)
                if g >= 1:
                    do_p2(g - 1, kts[g - 1])
            do_p2(ST // GRP - 1, kts[ST // GRP - 1])
            # r = colsum(Qi): the 32 row-matmuls double as PE filler while
            # the DVE/ACT evict the MT banks in parallel.
            for st in range(ST):
                nc.tensor.matmul(
                    r_ps[:], ones1[:], Qi_sb[:, ts(st, P)],
                    start=(st == 0), stop=(st == ST - 1),
                )
            nc.vector.tensor_copy(MT_sb[0][:], mt_ps[0][:])
            nc.scalar.activation(MT_sb[1][:], mt_ps[1][:], Copy)
            nc.vector.tensor_copy(r_sb[0:1, :], r_ps[:])
            mtps.release()
            qips.release()
            kpool.release()
            qpool.release()

            # ---- phase B: numT (P3) -> exp -> GT (P5) + s, per k-tile ----
            # Pipelined one group apart so GT(g) never waits on the exp of
            # its own group.  Within a numT region the rank-1 bias (bk x r)
            # runs LAST so r's eviction stays off the critical path.
            wkpool = tc.alloc_tile_pool(name="wkp", bufs=3)
            vpool = tc.alloc_tile_pool(name="vp", bufs=3)
            ntps = tc.alloc_tile_pool(name="ntps", bufs=3, space="PSUM")
            gtps = tc.alloc_tile_pool(name="gtps", bufs=1, space="PSUM")
            gt_ps = [gtps.tile([P, W4], f32, name=f"gt{h}") for h in range(2)]
            s_ps = gtps.tile([P, 1], f32, name="sps")
            for h in range(2):
                nc.tensor.matmul(gt_ps[h][:], zpad[:, 0:P], zpad[:],
                                 start=True, stop=False)

            def do_nt(g, wk):
                ntb = ntps.tile([P, W4], f32, tag="nt", name=f"nt{g}")
                for j in range(GRP):
                    ktile = g * GRP + j
                    for ct in range(CT):
                        nc.tensor.matmul(
                            ntb[:, ts(j, P)], wk[:, j, ct, :],
                            MT_sb[ct // GRP][:, ts(ct % GRP, P)],
                            start=(ct == 0), stop=False,
                        )
                    nc.tensor.matmul(
                        ntb[:, ts(j, P)], bk_sb[0:32, ts(ktile, P)],
                        r_sb[0:32, :], start=False, stop=True,
                    )
                nc.scalar.activation(
                    ET_sb[:, ts(g, W4)], ntb[:], Exp, scale=SCALE,
                )

            def do_gt(g, vt):
                for j in range(GRP):
                    ktile = g * GRP + j
                    for ct in range(CT):
                        nc.tensor.matmul(
                            gt_ps[ct // GRP][:, ts(ct % GRP, P)],
                            vt[:, j, ts(ct, P)], ET_sb[:, ts(ktile, P)],
                            start=False,
                            stop=(ktile == KT - 1 and ct % GRP == GRP - 1),
                        )
                    nc.tensor.matmul(
                        s_ps[:], ET_sb[:, ts(ktile, P)], onesS[:],
                        start=(ktile == 0), stop=(ktile == KT - 1),
                    )

            vts = {}
            for g in range(KT // GRP):
                wk = wkpool.tile([P, GRP, CT, P], e3, tag="wk", name=f"wk{g}")
                nc.sync.dma_start(wk[:], wkT[g])
                vt = vpool.tile([P, GRP, DIN], e3, tag="vt", name=f"vt{g}")
                nc.sync.dma_start(vt[:], vT[g])
                vts[g] = vt
                do_nt(g, wk)
                if g >= 1:
                    do_gt(g - 1, vts[g - 1])
            # final group bank-major: bank 0 finishes (and evicts on DVE)
            # while bank 1's matmuls still run; bank 1 evicts on ACT.
            gl = KT // GRP - 1
            vt = vts[gl]
            for h in range(2):
                for j in range(GRP):
                    ktile = gl * GRP + j
                    for ct in range(4 * h, 4 * h + 4):
                        nc.tensor.matmul(
                            gt_ps[h][:, ts(ct % GRP, P)],
                            vt[:, j, ts(ct, P)], ET_sb[:, ts(ktile, P)],
                            start=False,
                            stop=(j == GRP - 1 and ct % GRP == GRP - 1),
                        )
                if h == 0:
                    nc.vector.tensor_copy(GT_sb[0][:], gt_ps[0][:])
            for j in range(GRP):
                ktile = gl * GRP + j
                nc.tensor.matmul(
                    s_ps[:], ET_sb[:, ts(ktile, P)], onesS[:],
                    start=(ktile == 0), stop=(ktile == KT - 1),
                )
            nc.scalar.activation(GT_sb[1][:], gt_ps[1][:], Copy)
            nc.vector.reciprocal(rec_sb[:], s_ps[:])
            gtps.release()
            ntps.release()
            vpool.release()
            wkpool.release()

            if DEBUG:
                nc.scalar.dma_start(qi_dbg, Qi_sb[:])
                nc.scalar.dma_start(mt_dbg, MT_sb[:])
                nc.scalar.dma_start(r_dbg, r_sb[0:1, :])
                nc.scalar.dma_start(et_dbg, ET_sb[:])
                nc.scalar.dma_start(gt_dbg, GT_sb[:])
                nc.scalar.dma_start(rec_dbg, rec_sb[:])

            # ---- phase C: out = (GT.T @ WvT) * (1/s) ----
            with tc.tile_pool(name="wvp", bufs=3) as wvpool, \
                 tc.tile_pool(name="ops", bufs=4, space="PSUM") as ops, \
                 tc.tile_pool(name="otp", bufs=4) as otp:
                for b in range(CT):
                    wv = wvpool.tile([P, CT, 512], e3, tag="wv")
                    nc.sync.dma_start(wv[:], wvT[b])
                    if b == CT - 1:
                        # halve the final bank end-to-end (separate tiles:
                        # shared tiles would serialize on false WAR deps)
                        for hh in range(2):
                            oph = ops.tile([P, 256], f32, tag="oph",
                                           name=f"oph{hh}")
                            oth = otp.tile([P, 256], bf16, tag="oth",
                                           name=f"oth{hh}")
                            for ct in range(CT):
                                nc.tensor.matmul(
                                    oph[:], GT_sb[ct // GRP][:, ts(ct % GRP, P)],
                                    wv[:, ct, ts(hh, 256)],
                                    start=(ct == 0), stop=(ct == CT - 1),
                                )
                            nc.scalar.activation(
                                oth[:], oph[:], Copy, scale=rec_sb[:])
                            nc.scalar.dma_start(
                                p_out[:, b, ts(hh, 256)], oth[:])
                    else:
                        op = ops.tile([P, 512], f32, tag="op")
                        ot = otp.tile([P, 512], bf16, tag="ot")
                        for ct in range(CT):
                            nc.tensor.matmul(
                                op[:], GT_sb[ct // GRP][:, ts(ct % GRP, P)],
                                wv[:, ct, :],
                                start=(ct == 0), stop=(ct == CT - 1),
                            )
                        nc.scalar.activation(ot[:], op[:], Copy, scale=rec_sb[:])
                        nc.scalar.dma_start(p_out[:, b, :], ot[:])

    nc.compile()
    return nc


def _e3(a):
    return np.ascontiguousarray(a.astype(ml_dtypes.float8_e3m4))


def _bf16(a):
    return np.ascontiguousarray(a.astype(ml_dtypes.bfloat16))


def make_in_maps(q, k, v, Wq, bq, Wk, bk, Wv, bv):
    """Host-side shard + layout prep. Returns per-core input dicts."""
    f32 = np.float32
    q, k, v = (np.asarray(a, f32) for a in (q, k, v))
    Wq, bq, Wk, bk, Wv = (np.asarray(a, f32) for a in (Wq, bq, Wk, bk, Wv))

    # qT: q.T [din, s] -> [g, din-part, st-in-g, ct, s-in-tile]
    qt = q.T.reshape(CT, P, ST, P).transpose(2, 1, 0, 3)      # [st, dp, ct, s]
    qT = _e3(qt.reshape(ST // GRP, GRP, P, CT, P).transpose(0, 2, 1, 3, 4))
    # kT / vT: row-major [g, s-part, st-in-g, din]
    kT = _e3(k.reshape(ST // GRP, GRP, P, DIN).transpose(0, 2, 1, 3))
    vT = _e3(v.reshape(KT // GRP, GRP, P, DIN).transpose(0, 2, 1, 3))
    # wkT: Wk.T*WS [din, k] -> [g, din-part, kt-in-g, ct, k-in-tile]
    wkt = (Wk.T * WS).reshape(CT, P, KT, P).transpose(2, 1, 0, 3)
    wkT = _e3(wkt.reshape(KT // GRP, GRP, P, CT, P).transpose(0, 2, 1, 3, 4))
    # wvT: Wv.T*WS [din, dk] -> [b, din-part, ct, 512]
    wvT = _e3((Wv.T * WS).reshape(CT, P, CT, 512).transpose(2, 1, 0, 3))
    bkr = _bf16(bk * WS)

    in_maps = []
    for i in range(NCORES):
        sl = slice(i * QSH, (i + 1) * QSH)
        wq_i = _bf16((Wq[sl].T * WS).reshape(CT, P, QSH).transpose(1, 0, 2))
        bq_i = _bf16(np.tile(bq[sl] * WS, GRP))
        in_maps.append({
            "qT": qT, "kT": kT, "vT": vT, "wkT": wkT, "wvT": wvT,
            "wqT": wq_i, "bq": bq_i, "bk": bkr,
        })
    return in_maps


def combine(results, bv):
    """Host-side unshard: concatenate row shards, add bv."""
    out = np.concatenate(
        [r["p_out"].reshape(QSH, DK).astype(np.float32) for r in results],
        axis=0,
    )
    return out + np.asarray(bv, np.float32)[None, :]


def get_nc():
    with _lock:
        if "nc" not in _cache:
            _cache["nc"] = _build_module()
        return _cache["nc"]


def _run_spmd(in_maps):
    """Execute on the 8 NeuronCores.

    Under axon this mirrors bass_utils.run_bass_kernel_spmd's redirect
    (bass2jax.run_bass_via_pjrt) with two wall-clock fixes: the jitted
    executable is cached across calls, and core-replicated inputs use a
    replicated sharding instead of an 8x host-side concat.
    """
    from concourse._compat import axon_active
    from concourse import bass_utils

    nc = get_nc()
    if not axon_active():
        res = bass_utils.run_bass_kernel_spmd(nc, in_maps, list(range(NCORES)))
        return res.results
    r = _get_axon_runner(nc)
    return r.unpack(r.fn(*r.pack(in_maps)))


_SHARED = ("qT", "kT", "vT", "wkT", "wvT", "bk")  # identical on every core


class _AxonRunner:
    def __init__(self, nc, donate):
        import jax
        import numpy as _np
        from jax.sharding import Mesh, PartitionSpec, NamedSharding
        from jax.experimental.shard_map import shard_map
        import concourse.mybir as mybir
        from concourse import bass2jax

        bass2jax.install_neuronx_cc_hook()
        pname = nc.partition_id_tensor.name if nc.partition_id_tensor else None

        self.in_names, self.out_names, out_avals, self.zero_outs = [], [], [], []
        for alloc in nc.m.functions[0].allocations:
            if not isinstance(alloc, mybir.MemoryLocationSet):
                continue
            name = alloc.memorylocations[0].name
            if alloc.kind == "ExternalInput":
                if name != pname:
                    self.in_names.append(name)
            elif alloc.kind == "ExternalOutput":
                shape = tuple(alloc.tensor_shape)
                dtype = mybir.dt.np(alloc.dtype)
                self.out_names.append(name)
                out_avals.append(jax.core.ShapedArray(shape, dtype))
                self.zero_outs.append(_np.zeros(shape, dtype))
        self.out_avals = out_avals
        n_params = len(self.in_names)
        n_outs = len(out_avals)
        all_in_names = list(self.in_names) + list(self.out_names)
        if pname is not None:
            all_in_names.append(pname)

        def _body(*args):
            operands = list(args)
            if pname is not None:
                operands.append(bass2jax.partition_id_tensor())
            outs = bass2jax._bass_exec_p.bind(
                *operands,
                out_avals=tuple(out_avals),
                in_names=tuple(all_in_names),
                out_names=tuple(self.out_names),
                lowering_input_output_aliases=(),
                sim_require_finite=True,
                sim_require_nnan=True,
                nc=nc,
            )
            return tuple(outs)

        devices = jax.devices()[:NCORES]
        self.mesh = Mesh(_np.asarray(devices), ("core",))
        rep, sh = PartitionSpec(), PartitionSpec("core")
        self.in_specs = tuple(
            rep if n in _SHARED else sh for n in self.in_names
        ) + (sh,) * n_outs
        out_specs = (sh,) * n_outs
        donate_argnums = (
            tuple(range(n_params, n_params + n_outs)) if donate else ()
        )
        self.fn = jax.jit(
            shard_map(_body, mesh=self.mesh, in_specs=self.in_specs,
                      out_specs=out_specs, check_rep=False),
            donate_argnums=donate_argnums, keep_unused=True,
        )
        self._jax = jax
        self._NamedSharding = NamedSharding

    def pack(self, in_maps):
        import numpy as _np
        args = []
        for name in self.in_names:
            if name in _SHARED:
                args.append(_np.asarray(in_maps[0][name]))
            else:
                args.append(
                    _np.concatenate(
                        [_np.asarray(m[name]) for m in in_maps], axis=0)
                )
        for z in self.zero_outs:
            args.append(_np.zeros((NCORES * z.shape[0], *z.shape[1:]), z.dtype))
        return args

    def to_device(self, args):
        """Pre-place packed args with their shardings (for timing loops)."""
        return [
            self._jax.device_put(
                a, self._NamedSharding(self.mesh, spec))
            for a, spec in zip(args, self.in_specs)
        ]

    def unpack(self, out_arrs):
        import numpy as _np
        return [
            {
                name: _np.asarray(out_arrs[i]).reshape(
                    NCORES, *self.out_avals[i].shape)[c]
                for i, name in enumerate(self.out_names)
            }
            for c in range(NCORES)
        ]


def _get_axon_runner(nc, donate=False):
    """Cached executable; donate=False keeps output operands reusable
    across calls (legal here: the kernel writes every output element,
    so nothing reads the pre-zeroed buffers)."""
    key = ("runner", donate)
    with _lock:
        if key in _cache:
            return _cache[key]
    runner = _AxonRunner(nc, donate)
    with _lock:
        _cache[key] = runner
    return runner


def kernel(q, k, v, Wq, bq, Wk, bk, Wv, bv):
    q, k, v, Wq, bq, Wk, bk, Wv, bv = (
        np.asarray(a) for a in (q, k, v, Wq, bq, Wk, bk, Wv, bv))
    in_maps = make_in_maps(q, k, v, Wq, bq, Wk, bk, Wv, bv)
    results = _run_spmd(in_maps)
    return combine(results, np.asarray(bv))


# revision 13
# speedup vs baseline: 1.0608x; 1.0597x over previous
"""CrossAttentionHead kernel for 8x TRN2 NeuronCores (Bass/Tile).

Reference computation (all fp32):
    Q = q @ Wq.T + bq          # [S, DQ]      S=4096, DQ=1024
    K = k @ Wk.T + bk          # [S, DK]      DK=4096
    V = v @ Wv.T + bv          # [S, DK]
    num  = Q.T @ K             # [DQ, DK]
    attn = softmax(num / sqrt(DK), axis=-1)
    out  = attn @ V            # [DQ, DK]   (contraction uses S == DK)

Algebraic restructure (the key optimization): K and V are never
materialized.  With M = Q.T @ k  ([DQ, DIN]):
    num  = M @ Wk.T + (Q.T @ 1) x bk          (bias is rank-1)
    out  = (attn @ v) @ Wv.T + bv             (attn rows sum to 1)
This cuts the FLOPs 4x (146 -> 36.5 GFLOP total) and shards cleanly by
ROWS of the output: core i owns DQ rows [i*128, (i+1)*128).  Per core:
    Qi   = q @ Wq_i.T + bq_i                  # [S, 128]
    MT   = k.T-chunks @ Qi                    # M_i.T  [DIN, 128]
    numT = Wk-chunks @ MT + bk x r            # [DK, 128], r = colsum(Qi)
    ET   = exp(numT / 64)                     # softmax numerator (no max
                                              # subtraction: |num/64| < ~3)
    s    = ET.T @ 1                           # [128, 1] denominators
    GT   = v.T-chunks @ ET                    # (E @ v).T  [DIN, 128]
    out  = (GT.T @ Wv.T) * (1/s)              # [128, DK]
Every matmul lands in its natural [K-on-partitions] layout -- zero
transposes.  No cross-core communication; the host concatenates the 8
row shards and adds bv.

Precision: HBM loads are float8_e3m4 (q, k, v, Wk.T, Wv.T; weights
pre-scaled x64 so entries ~U(-1,1) stay in e3m4's normal range),
intermediates bf16, PSUM/bias fp32.  Numpy-sim max rel err ~1.3e-2
(gate 2e-2).  e3m4 halves DMA bytes (21 MB/core) which would otherwise
be the bottleneck (360 GB/s/core aggregate in the cost model).
"""

import threading

import numpy as np
import ml_dtypes

S = 4096
DIN = 1024
DQ = 1024
DK = 4096
NCORES = 8
P = 128
QSH = DQ // NCORES          # 128: per-core shard of DQ rows
ST = S // P                 # 32 s-tiles
CT = DIN // P               # 8 contraction chunks over DIN
KT = DK // P                # 32 k-tiles
GRP = 4                     # DMA grouping: 4 tiles per transfer
WS = 64.0                   # weight prescale (e3m4 range/subnormal fix)
SCALE = 1.0 / (64.0 * WS * WS)   # exp arg = numT_scaled * SCALE

DEBUG = False   # adds intermediate-dump outputs to the module

_lock = threading.Lock()
_cache = {}


def _build_module():
    import concourse.bacc as bacc
    import concourse.mybir as mybir
    import concourse.tile as tile

    bf16 = mybir.dt.bfloat16
    f32 = mybir.dt.float32
    e3 = mybir.dt.float8e3
    Exp = mybir.ActivationFunctionType.Exp
    Copy = mybir.ActivationFunctionType.Copy

    nc = bacc.Bacc(
        "TRN2", target_bir_lowering=False, debug=False, num_devices=NCORES
    )

    # [g, din-part, st-in-g, ct, s]: lhsT tiles for P1
    qT = nc.dram_tensor("qT", [ST // GRP, P, GRP, CT, P], e3,
                        kind="ExternalInput").ap()
    # [g, s-part, st-in-g, din]: lhsT tiles for P2 (k natural row-major)
    kT = nc.dram_tensor("kT", [ST // GRP, P, GRP, DIN], e3,
                        kind="ExternalInput").ap()
    # [g, k-part, kt-in-g, din]: lhsT tiles for P5 (v natural row-major)
    vT = nc.dram_tensor("vT", [KT // GRP, P, GRP, DIN], e3,
                        kind="ExternalInput").ap()
    # [g, din-part, kt-in-g, ct, k]: lhsT tiles for P3
    wkT = nc.dram_tensor("wkT", [KT // GRP, P, GRP, CT, P], e3,
                         kind="ExternalInput").ap()
    # [b, din-part, ct, 512]: moving tiles for P6
    wvT = nc.dram_tensor("wvT", [CT, P, CT, 512], e3,
                         kind="ExternalInput").ap()
    # [din-part, ct, q]: moving tile for P1 (per-core shard, x WS)
    wqT = nc.dram_tensor("wqT", [P, CT, P], bf16, kind="ExternalInput").ap()
    bq = nc.dram_tensor("bq", [GRP * P], bf16, kind="ExternalInput").ap()  # x WS, tiled 4x
    bk = nc.dram_tensor("bk", [DK], bf16, kind="ExternalInput").ap()    # x WS
    p_out = nc.dram_tensor("p_out", [P, CT, 512], bf16,
                           kind="ExternalOutput").ap()
    if DEBUG:
        qi_dbg = nc.dram_tensor("qi_dbg", [P, ST * P], bf16,
                                kind="ExternalOutput").ap()
        mt_dbg = nc.dram_tensor("mt_dbg", [P, DIN], bf16,
                                kind="ExternalOutput").ap()
        r_dbg = nc.dram_tensor("r_dbg", [1, P], bf16,
                               kind="ExternalOutput").ap()
        et_dbg = nc.dram_tensor("et_dbg", [P, KT * P], bf16,
                                kind="ExternalOutput").ap()
        gt_dbg = nc.dram_tensor("gt_dbg", [P, DIN], bf16,
                                kind="ExternalOutput").ap()
        rec_dbg = nc.dram_tensor("rec_dbg", [P, 1], f32,
                                 kind="ExternalOutput").ap()

    def ts(i, sz):
        return slice(i * sz, (i + 1) * sz)

    W4 = GRP * P  # 512: one PSUM bank of fp32

    with tile.TileContext(nc) as tc:
        with tc.tile_pool(name="persist", bufs=1) as persist:
            WqT_sb = persist.tile([P, CT, P], bf16)
            Bq = persist.tile([P, W4], bf16)          # bq_i tiled 4x
            bk_sb = persist.tile([32, DK], bf16)   # row 0 = bk*WS, rows 1-31 zero
                                                   # (K=1 matmuls round tile_size
                                                   # up to 32 partitions on HW)
            ones1 = persist.tile([P, 1], bf16)
            onesS = persist.tile([P, 1], bf16)
            Qi_sb = persist.tile([P, ST * P], bf16)   # 8 KB/part
            MT_sb = [persist.tile([P, W4], bf16, name=f"MT{h}")
                     for h in range(2)]
            ET_sb = persist.tile([P, KT * P], bf16)   # 8 KB/part
            GT_sb = [persist.tile([P, W4], bf16, name=f"GT{h}")
                     for h in range(2)]
            r_sb = persist.tile([32, P], bf16)     # row 0 = r, rows 1-31 zero
            rec_sb = persist.tile([P, 1], f32)
            nc.vector.memset(ones1[:], 1.0)
            nc.vector.memset(onesS[:], WS)
            nc.vector.memset(bk_sb[:], 0.0)
            nc.vector.memset(r_sb[:], 0.0)

            # All load DMAs go on the sync (SP) queue in need-order; the
            # pool buffer rotation (bufs=3) head-of-line-blocks the queue
            # so later tensors cannot steal DMA bandwidth early.

            qpool = tc.alloc_tile_pool(name="qp", bufs=3)
            kpool = tc.alloc_tile_pool(name="kp", bufs=3)
            wkpool = tc.alloc_tile_pool(name="wkp", bufs=3)
            vpool = tc.alloc_tile_pool(name="vp", bufs=3)
            wvpool = tc.alloc_tile_pool(name="wvp", bufs=4)
            otp = tc.alloc_tile_pool(name="otp", bufs=4)
            qips = tc.alloc_tile_pool(name="qips", bufs=2, space="PSUM")
            mtps = tc.alloc_tile_pool(name="mtps", bufs=1, space="PSUM")
            mt_ps = [mtps.tile([P, W4], f32, name=f"mt{h}") for h in range(2)]
            r_ps = mtps.tile([1, P], f32, name="rps")

            # ---- phase A: Qi projection (P1) + MT = k.T @ Qi (P2) ----
            # PSUM accumulators are packed 4-per-bank; evictions and the
            # bias add run on whole [P, 512] banks.  A start=True matmul
            # marks its entire 2KB zero region (= the bank) pending-zero,
            # so packed banks are opened ONCE by a zeroing matmul and all
            # real accumulation runs start=False with a single final stop.
            # P1/P2 are software-pipelined one group apart so P2(g) never
            # waits on the DVE eviction of Qi(g).
            def do_p1(g, qt):
                qp = qips.tile([P, W4], f32, tag="qip", name=f"qp{g}")
                for j in range(GRP):
                    for ct in range(CT):
                        nc.tensor.matmul(
                            qp[:, ts(j, P)], qt[:, j, ct, :], WqT_sb[:, ct, :],
                            start=(ct == 0), stop=(ct == CT - 1),
                        )
                nc.vector.tensor_add(Qi_sb[:, ts(g, W4)], qp[:], Bq[:])

            def do_p2(g, kt):
                for j in range(GRP):
                    st = g * GRP + j
                    for ct in range(CT):
                        nc.tensor.matmul(
                            mt_ps[ct // GRP][:, ts(ct % GRP, P)],
                            kt[:, j, ts(ct, P)], Qi_sb[:, ts(st, P)],
                            start=(st == 0 and ct % GRP == 0),
                            stop=(st == ST - 1 and ct % GRP == GRP - 1),
                        )

            kts = {}
            for g in range(ST // GRP):
                qt = qpool.tile([P, GRP, CT, P], e3, tag="qt", name=f"qt{g}")
                if g == 0:
                    # split the first load so P1 starts earlier
                    nc.sync.dma_start(qt[:, 0:2], qT[0][:, 0:2])
                    nc.sync.dma_start(WqT_sb[:], wqT)
                    nc.sync.dma_start(Bq[:], bq.unsqueeze(0).to_broadcast((P, W4)))
                    nc.sync.dma_start(qt[:, 2:4], qT[0][:, 2:4])
                else:
                    nc.sync.dma_start(qt[:], qT[g])
                kt = kpool.tile([P, GRP, DIN], e3, tag="kt", name=f"kt{g}")
                nc.sync.dma_start(kt[:], kT[g])
                if g == 0:
                    nc.sync.dma_start(bk_sb[0:1, :], bk.unsqueeze(0))
                kts[g] = kt
                do_p1(g, qt)
                if g >= 1:
                    do_p2(g - 1, kts[g - 1])
            do_p2(ST // GRP - 1, kts[ST // GRP - 1])
            # r = colsum(Qi): the 32 row-matmuls double as PE filler while
            # the DVE/ACT evict the MT banks in parallel.
            for st in range(ST):
                nc.tensor.matmul(
                    r_ps[:], ones1[:], Qi_sb[:, ts(st, P)],
                    start=(st == 0), stop=(st == ST - 1),
                )
            nc.vector.tensor_copy(MT_sb[0][:], mt_ps[0][:])
            nc.scalar.activation(MT_sb[1][:], mt_ps[1][:], Copy)
            nc.vector.tensor_copy(r_sb[0:1, :], r_ps[:])
            mtps.release()
            qips.release()

            # ---- phase B: numT (P3) -> exp -> GT (P5) + s, per k-tile ----
            # Pipelined one group apart so GT(g) never waits on the exp of
            # its own group.  Within a numT region the rank-1 bias (bk x r)
            # runs LAST so r's eviction stays off the critical path.
            ntps = tc.alloc_tile_pool(name="ntps", bufs=3, space="PSUM")
            gtps = tc.alloc_tile_pool(name="gtps", bufs=1, space="PSUM")
            gt_ps = [gtps.tile([P, W4], f32, name=f"gt{h}") for h in range(2)]
            s_ps = gtps.tile([P, 1], f32, name="sps")
            def do_nt(g, wk):
                ntb = ntps.tile([P, W4], f32, tag="nt", name=f"nt{g}")
                for j in range(GRP):
                    ktile = g * GRP + j
                    for ct in range(CT):
                        nc.tensor.matmul(
                            ntb[:, ts(j, P)], wk[:, j, ct, :],
                            MT_sb[ct // GRP][:, ts(ct % GRP, P)],
                            start=(ct == 0), stop=False,
                        )
                    nc.tensor.matmul(
                        ntb[:, ts(j, P)], bk_sb[0:32, ts(ktile, P)],
                        r_sb[0:32, :], start=False, stop=True,
                    )
                nc.scalar.activation(
                    ET_sb[:, ts(g, W4)], ntb[:], Exp, scale=SCALE,
                )

            def do_gt(g, vt):
                for j in range(GRP):
                    ktile = g * GRP + j
                    for ct in range(CT):
                        nc.tensor.matmul(
                            gt_ps[ct // GRP][:, ts(ct % GRP, P)],
                            vt[:, j, ts(ct, P)], ET_sb[:, ts(ktile, P)],
                            start=(ktile == 0 and ct % GRP == 0),
                            stop=(ktile == KT - 1 and ct % GRP == GRP - 1),
                        )
                    nc.tensor.matmul(
                        s_ps[:], ET_sb[:, ts(ktile, P)], onesS[:],
                        start=(ktile == 0), stop=(ktile == KT - 1),
                    )

            vts = {}
            for g in range(KT // GRP):
                wk = wkpool.tile([P, GRP, CT, P], e3, tag="wk", name=f"wk{g}")
                nc.sync.dma_start(wk[:], wkT[g])
                vt = vpool.tile([P, GRP, DIN], e3, tag="vt", name=f"vt{g}")
                nc.sync.dma_start(vt[:], vT[g])
                vts[g] = vt
                do_nt(g, wk)
                if g >= 1:
                    do_gt(g - 1, vts[g - 1])
            # final group bank-major: bank 0 finishes (and evicts on DVE)
            # while bank 1's matmuls still run; bank 1 evicts on ACT.
            gl = KT // GRP - 1
            vt = vts[gl]
            for h in range(2):
                for j in range(GRP):
                    ktile = gl * GRP + j
                    for ct in range(4 * h, 4 * h + 4):
                        nc.tensor.matmul(
                            gt_ps[h][:, ts(ct % GRP, P)],
                            vt[:, j, ts(ct, P)], ET_sb[:, ts(ktile, P)],
                            start=False,
                            stop=(j == GRP - 1 and ct % GRP == GRP - 1),
                        )
                if h == 0:
                    nc.vector.tensor_copy(GT_sb[0][:], gt_ps[0][:])
            for j in range(GRP):
                ktile = gl * GRP + j
                nc.tensor.matmul(
                    s_ps[:], ET_sb[:, ts(ktile, P)], onesS[:],
                    start=(ktile == 0), stop=(ktile == KT - 1),
                )
            nc.scalar.activation(GT_sb[1][:], gt_ps[1][:], Copy)
            nc.vector.reciprocal(rec_sb[:], s_ps[:])
            gtps.release()
            ntps.release()

            if DEBUG:
                nc.scalar.dma_start(qi_dbg, Qi_sb[:])
                nc.scalar.dma_start(mt_dbg, MT_sb[:])
                nc.scalar.dma_start(r_dbg, r_sb[0:1, :])
                nc.scalar.dma_start(et_dbg, ET_sb[:])
                nc.scalar.dma_start(gt_dbg, GT_sb[:])
                nc.scalar.dma_start(rec_dbg, rec_sb[:])

            # ---- phase C: out = (GT.T @ WvT) * (1/s) ----
            with tc.tile_pool(name="ops", bufs=4, space="PSUM") as ops:
                for b in range(CT):
                    wv = wvpool.tile([P, CT, 512], e3, tag="wv")
                    nc.sync.dma_start(wv[:], wvT[b])
                    if b == CT - 1:
                        # halve the final bank end-to-end (separate tiles:
                        # shared tiles would serialize on false WAR deps)
                        for hh in range(2):
                            oph = ops.tile([P, 256], f32, tag="oph",
                                           name=f"oph{hh}")
                            oth = otp.tile([P, 256], bf16, tag="oth",
                                           name=f"oth{hh}")
                            for ct in range(CT):
                                nc.tensor.matmul(
                                    oph[:], GT_sb[ct // GRP][:, ts(ct % GRP, P)],
                                    wv[:, ct, ts(hh, 256)],
                                    start=(ct == 0), stop=(ct == CT - 1),
                                )
                            nc.scalar.activation(
                                oth[:], oph[:], Copy, scale=rec_sb[:])
                            nc.sync.dma_start(
                                p_out[:, b, ts(hh, 256)], oth[:])
                    else:
                        op = ops.tile([P, 512], f32, tag="op")
                        ot = otp.tile([P, 512], bf16, tag="ot")
                        for ct in range(CT):
                            nc.tensor.matmul(
                                op[:], GT_sb[ct // GRP][:, ts(ct % GRP, P)],
                                wv[:, ct, :],
                                start=(ct == 0), stop=(ct == CT - 1),
                            )
                        nc.scalar.activation(ot[:], op[:], Copy, scale=rec_sb[:])
                        nc.sync.dma_start(p_out[:, b, :], ot[:])
            for pool in (otp, wvpool, vpool, wkpool, kpool, qpool):
                pool.release()

    nc.compile()
    return nc


def _e3(a):
    return np.ascontiguousarray(a.astype(ml_dtypes.float8_e3m4))


def _bf16(a):
    return np.ascontiguousarray(a.astype(ml_dtypes.bfloat16))


def make_in_maps(q, k, v, Wq, bq, Wk, bk, Wv, bv):
    """Host-side shard + layout prep. Returns per-core input dicts."""
    f32 = np.float32
    q, k, v = (np.asarray(a, f32) for a in (q, k, v))
    Wq, bq, Wk, bk, Wv = (np.asarray(a, f32) for a in (Wq, bq, Wk, bk, Wv))

    # qT: q.T [din, s] -> [g, din-part, st-in-g, ct, s-in-tile]
    qt = q.T.reshape(CT, P, ST, P).transpose(2, 1, 0, 3)      # [st, dp, ct, s]
    qT = _e3(qt.reshape(ST // GRP, GRP, P, CT, P).transpose(0, 2, 1, 3, 4))
    # kT / vT: row-major [g, s-part, st-in-g, din]
    kT = _e3(k.reshape(ST // GRP, GRP, P, DIN).transpose(0, 2, 1, 3))
    vT = _e3(v.reshape(KT // GRP, GRP, P, DIN).transpose(0, 2, 1, 3))
    # wkT: Wk.T*WS [din, k] -> [g, din-part, kt-in-g, ct, k-in-tile]
    wkt = (Wk.T * WS).reshape(CT, P, KT, P).transpose(2, 1, 0, 3)
    wkT = _e3(wkt.reshape(KT // GRP, GRP, P, CT, P).transpose(0, 2, 1, 3, 4))
    # wvT: Wv.T*WS [din, dk] -> [b, din-part, ct, 512]
    wvT = _e3((Wv.T * WS).reshape(CT, P, CT, 512).transpose(2, 1, 0, 3))
    bkr = _bf16(bk * WS)

    in_maps = []
    for i in range(NCORES):
        sl = slice(i * QSH, (i + 1) * QSH)
        wq_i = _bf16((Wq[sl].T * WS).reshape(CT, P, QSH).transpose(1, 0, 2))
        bq_i = _bf16(np.tile(bq[sl] * WS, GRP))
        in_maps.append({
            "qT": qT, "kT": kT, "vT": vT, "wkT": wkT, "wvT": wvT,
            "wqT": wq_i, "bq": bq_i, "bk": bkr,
        })
    return in_maps


def combine(results, bv):
    """Host-side unshard: concatenate row shards, add bv."""
    out = np.concatenate(
        [r["p_out"].reshape(QSH, DK).astype(np.float32) for r in results],
        axis=0,
    )
    return out + np.asarray(bv, np.float32)[None, :]


def get_nc():
    with _lock:
        if "nc" not in _cache:
            _cache["nc"] = _build_module()
        return _cache["nc"]


def _run_spmd(in_maps):
    """Execute on the 8 NeuronCores.

    Under axon this mirrors bass_utils.run_bass_kernel_spmd's redirect
    (bass2jax.run_bass_via_pjrt) with two wall-clock fixes: the jitted
    executable is cached across calls, and core-replicated inputs use a
    replicated sharding instead of an 8x host-side concat.
    """
    from concourse._compat import axon_active
    from concourse import bass_utils

    nc = get_nc()
    if not axon_active():
        res = bass_utils.run_bass_kernel_spmd(nc, in_maps, list(range(NCORES)))
        return res.results
    r = _get_axon_runner(nc)
    return r.unpack(r.fn(*r.pack(in_maps)))


_SHARED = ("qT", "kT", "vT", "wkT", "wvT", "bk")  # identical on every core


class _AxonRunner:
    def __init__(self, nc, donate):
        import jax
        import numpy as _np
        from jax.sharding import Mesh, PartitionSpec, NamedSharding
        from jax.experimental.shard_map import shard_map
        import concourse.mybir as mybir
        from concourse import bass2jax

        bass2jax.install_neuronx_cc_hook()
        pname = nc.partition_id_tensor.name if nc.partition_id_tensor else None

        self.in_names, self.out_names, out_avals, self.zero_outs = [], [], [], []
        for alloc in nc.m.functions[0].allocations:
            if not isinstance(alloc, mybir.MemoryLocationSet):
                continue
            name = alloc.memorylocations[0].name
            if alloc.kind == "ExternalInput":
                if name != pname:
                    self.in_names.append(name)
            elif alloc.kind == "ExternalOutput":
                shape = tuple(alloc.tensor_shape)
                dtype = mybir.dt.np(alloc.dtype)
                self.out_names.append(name)
                out_avals.append(jax.core.ShapedArray(shape, dtype))
                self.zero_outs.append(_np.zeros(shape, dtype))
        self.out_avals = out_avals
        n_params = len(self.in_names)
        n_outs = len(out_avals)
        all_in_names = list(self.in_names) + list(self.out_names)
        if pname is not None:
            all_in_names.append(pname)

        def _body(*args):
            operands = list(args)
            if pname is not None:
                operands.append(bass2jax.partition_id_tensor())
            outs = bass2jax._bass_exec_p.bind(
                *operands,
                out_avals=tuple(out_avals),
                in_names=tuple(all_in_names),
                out_names=tuple(self.out_names),
                lowering_input_output_aliases=(),
                sim_require_finite=True,
                sim_require_nnan=True,
                nc=nc,
            )
            return tuple(outs)

        devices = jax.devices()[:NCORES]
        self.mesh = Mesh(_np.asarray(devices), ("core",))
        rep, sh = PartitionSpec(), PartitionSpec("core")
        self.in_specs = tuple(
            rep if n in _SHARED else sh for n in self.in_names
        ) + (sh,) * n_outs
        out_specs = (sh,) * n_outs
        donate_argnums = (
            tuple(range(n_params, n_params + n_outs)) if donate else ()
        )
        self.fn = jax.jit(
            shard_map(_body, mesh=self.mesh, in_specs=self.in_specs,
                      out_specs=out_specs, check_rep=False),
            donate_argnums=donate_argnums, keep_unused=True,
        )
        self._jax = jax
        self._NamedSharding = NamedSharding

    def pack(self, in_maps):
        import numpy as _np
        args = []
        for name in self.in_names:
            if name in _SHARED:
                args.append(_np.asarray(in_maps[0][name]))
            else:
                args.append(
                    _np.concatenate(
                        [_np.asarray(m[name]) for m in in_maps], axis=0)
                )
        for z in self.zero_outs:
            args.append(_np.zeros((NCORES * z.shape[0], *z.shape[1:]), z.dtype))
        return args

    def to_device(self, args):
        """Pre-place packed args with their shardings (for timing loops)."""
        return [
            self._jax.device_put(
                a, self._NamedSharding(self.mesh, spec))
            for a, spec in zip(args, self.in_specs)
        ]

    def unpack(self, out_arrs):
        import numpy as _np
        return [
            {
                name: _np.asarray(out_arrs[i]).reshape(
                    NCORES, *self.out_avals[i].shape)[c]
                for i, name in enumerate(self.out_names)
            }
            for c in range(NCORES)
        ]


def _get_axon_runner(nc, donate=False):
    """Cached executable; donate=False keeps output operands reusable
    across calls (legal here: the kernel writes every output element,
    so nothing reads the pre-zeroed buffers)."""
    key = ("runner", donate)
    with _lock:
        if key in _cache:
            return _cache[key]
    runner = _AxonRunner(nc, donate)
    with _lock:
        _cache[key] = runner
    return runner


def kernel(q, k, v, Wq, bq, Wk, bk, Wv, bv):
    q, k, v, Wq, bq, Wk, bk, Wv, bv = (
        np.asarray(a) for a in (q, k, v, Wq, bq, Wk, bk, Wv, bv))
    in_maps = make_in_maps(q, k, v, Wq, bq, Wk, bk, Wv, bv)
    results = _run_spmd(in_maps)
    return combine(results, np.asarray(bv))
